# revision 1
# baseline (speedup 1.0000x reference)
"""Trainium2 Bass kernel for CausalSelfAttention (GQA + QK-RMSNorm + RoPE +
sliding-window causal attention + out-proj), tensor-parallel over 8 NeuronCores.

Sharding: core i owns q heads 4i..4i+3 and kv group i (split of the qkv output
dim and the proj input dim). The QK RMSNorm spans ALL heads (norm over the full
flattened q/k vectors), so per-core partial sums of squares are combined with a
tiny (2x512 f32 per t-tile) AllReduce. The proj contribution of each core is a
partial sum over its heads; partials are summed on the host.

Self-contained: hardcodes B=1, T=2048, C=4096, H=32, G=8, D=128, W=1024.
"""

import sys
import types
import numpy as np
import ml_dtypes

import concourse.bass as bass
import concourse.tile as tile
from concourse import bacc, mybir
from concourse import bass_utils
from concourse.bass import ts
from concourse.masks import make_identity

BF16 = ml_dtypes.bfloat16
FP32 = mybir.dt.float32
BF = mybir.dt.bfloat16

T = 2048          # tokens
C = 4096          # n_embd
D = 128           # head dim
HL = 4            # local q heads per core
QR = HL * D       # local q rows = 512
M = 6             # local qkv out chunks of 128 (4 q + 1 k + 1 v)
NCORES = 8
EPS = 1e-5
INV_SQRT_D = 1.0 / np.sqrt(128.0)
NT = 4            # t-tiles of 512
NSUB = 2          # sub-tiles of 256 per t-tile
NG = 8            # q groups of 256
NKV = 16          # kv blocks of 128
WINB = 8          # window = 8 kv blocks


def _install_ntff_hook():
    """Re-register the axon NTFF profiling hook (the image lacks
    antenv.axon_hooks, so boot() degraded silently)."""
    if "antenv.axon_hooks" in sys.modules:
        return
    mod = types.ModuleType("antenv.axon_hooks")
    holder = [None]
    mod.set_axon_ntff_profile_hook = lambda h: holder.__setitem__(0, h)
    mod.get_axon_ntff_profile_hook = lambda: holder[0]
    sys.modules["antenv.axon_hooks"] = mod
    try:
        import antenv
        antenv.axon_hooks = mod
        from trn_agent_boot.trn_boot import _ntff_profile_via_ctypes
        mod.set_axon_ntff_profile_hook(
            _ntff_profile_via_ctypes("/opt/axon/libaxon_pjrt.so"))
    except Exception:
        pass


_install_ntff_hook()


def build_program(stage=5):
    nc = bacc.Bacc("TRN2", target_bir_lowering=False, debug=False,
                   num_devices=NCORES)

    xt_d = nc.dram_tensor("xt", [8, 128, 32, 256], BF, kind="ExternalInput").ap()
    wq_d = nc.dram_tensor("wq", [128, 32, 768], BF, kind="ExternalInput").ap()
    wp_d = nc.dram_tensor("wp", [128, 4, C], BF, kind="ExternalInput").ap()
    cs_d = nc.dram_tensor("cs", [128, T], BF, kind="ExternalInput").ap()
    sn_d = nc.dram_tensor("sn", [128, T], BF, kind="ExternalInput").ap()
    iw2_d = nc.dram_tensor("iw2", [128, 8], BF, kind="ExternalInput").ap()
    pm_d = nc.dram_tensor("pm", [128, 128], BF, kind="ExternalInput").ap()
    yo_d = nc.dram_tensor("yo", [32, 128, T], FP32, kind="ExternalOutput").ap()

    with tile.TileContext(nc) as tc:
        _emit(nc, tc, xt_d, wq_d, wp_d, cs_d, sn_d, iw2_d, pm_d, yo_d, stage)
    nc.compile()
    return nc


def _emit(nc, tc, xt_d, wq_d, wp_d, cs_d, sn_d, iw2_d, pm_d, yo_d, stage=5):
    import contextlib
    ctx = contextlib.ExitStack()
    with ctx:
        # ---------------- pools ----------------
        const = ctx.enter_context(tc.tile_pool(name="const", bufs=1))
        persist = ctx.enter_context(tc.tile_pool(name="persist", bufs=1))
        xpool = ctx.enter_context(tc.tile_pool(name="xpool", bufs=2))
        qkvpool = ctx.enter_context(tc.tile_pool(name="qkvpool", bufs=3))
        qkvcopy = ctx.enter_context(tc.tile_pool(name="qkvcopy", bufs=2))
        sqpool = ctx.enter_context(tc.tile_pool(name="sqpool", bufs=2))
        stgpool = ctx.enter_context(tc.tile_pool(name="stgpool", bufs=2))
        rcv = ctx.enter_context(tc.tile_pool(name="rcv", bufs=2))
        rqbp = ctx.enter_context(tc.tile_pool(name="rqbp", bufs=2))
        ropep = ctx.enter_context(tc.tile_pool(name="ropep", bufs=1))
        cspool = ctx.enter_context(tc.tile_pool(name="cspool", bufs=2))
        ppool = ctx.enter_context(tc.tile_pool(name="ppool", bufs=12))
        dsbp = ctx.enter_context(tc.tile_pool(name="dsbp", bufs=2))
        drbp = ctx.enter_context(tc.tile_pool(name="drbp", bufs=2))
        yhatp = ctx.enter_context(tc.tile_pool(name="yhatp", bufs=2))
        ostgp = ctx.enter_context(tc.tile_pool(name="ostgp", bufs=2))
        dram = ctx.enter_context(tc.tile_pool(name="dram", bufs=2, space="DRAM"))

        # PSUM pools: 8 banks total
        mm2 = ctx.enter_context(tc.tile_pool(name="mm2", bufs=2, space="PSUM"))
        sumps = ctx.enter_context(tc.tile_pool(name="sumps", bufs=1, space="PSUM"))
        scps = ctx.enter_context(tc.tile_pool(name="scps", bufs=2, space="PSUM"))
        yps = ctx.enter_context(tc.tile_pool(name="yps", bufs=2, space="PSUM"))
        dps = ctx.enter_context(tc.tile_pool(name="dps", bufs=1, space="PSUM"))

        # ---------------- constants ----------------
        ident = const.tile([128, 128], BF)
        make_identity(nc, ident)
        ones = const.tile([128, 1], BF)
        nc.vector.memset(ones, 1.0)
        epsc = const.tile([33, 1], FP32)
        nc.vector.memset(epsc, EPS)
        zerob = const.tile([128, 1], FP32)
        nc.vector.memset(zerob, 0.0)
        iw2 = const.tile([128, 8], BF)
        nc.sync.dma_start(iw2, iw2_d)
        pm = const.tile([128, 128], BF)
        nc.sync.dma_start(pm, pm_d)
        # window/causal edge masks for m in {1, 0, -7, -8}; tile [kv=128, q=256]
        # allowed iff 0 <= d <= 1023 with d = q - kv - 128*m
        masks = {}
        for m in (1, 0, -7, -8):
            mk = const.tile([128, 256], BF, tag=f"mask{m}")
            nc.vector.memset(mk, 1.0)
            nc.gpsimd.affine_select(
                out=mk, in_=mk, compare_op=mybir.AluOpType.is_ge,
                fill=0.0, base=-128 * m, pattern=[[1, 256]],
                channel_multiplier=-1)
            nc.gpsimd.affine_select(
                out=mk, in_=mk, compare_op=mybir.AluOpType.is_ge,
                fill=0.0, base=128 * m + 1023, pattern=[[-1, 256]],
                channel_multiplier=1)
            masks[m] = mk

        # ---------------- persistent buffers ----------------
        wq_sb = persist.tile([128, 32, 768], BF)
        for h in range(4):
            nc.sync.dma_start(wq_sb[:, ts(h, 8), :], wq_d[:, ts(h, 8), :])
        wp_sb = persist.tile([128, 4, C], BF)
        for h in range(4):
            nc.scalar.dma_start(wp_sb[:, h, :], wp_d[:, h, :])

        qhat = persist.tile([128, HL, T], BF)     # roped+normed q (pre-scaled)
        khat = persist.tile([128, T], BF)         # roped k (norm in exp scale)
        v_sb = persist.tile([128, NKV, 128], BF)  # v transposed [kv, d]

        # ========= phase A: qkv (t) pipelined with epilogue (t-1) =========
        # Epilogue (v-transpose, sumsq+AllReduce, rstd, rope) for t-1 is
        # emitted after the qkv matmuls of t, so the PE never stalls on the
        # cross-engine epilogue chains and the collective hides under qkv.
        qkv_tiles = {}
        vstages = {}
        qkv_tiles2 = {}
        arouts = {}

        def emit_qkv(t):
            qkvt = qkvpool.tile([128, 5, 512], BF, tag="qkvt")
            vstage = qkvcopy.tile([128, 512], BF, tag="vstage")
            qkv_tiles[t] = qkvt
            vstages[t] = vstage
            for sub in range(NSUB):
                col0 = 512 * t + 256 * sub
                cslc = slice(256 * sub, 256 * sub + 256)
                sti = 2 * t + sub
                xa = xpool.tile([128, 16, 256], BF, tag="xa")
                nc.sync.dma_start(xa, xt_d[sti, :, 0:16, :])
                xb = xpool.tile([128, 16, 256], BF, tag="xb")
                nc.scalar.dma_start(xb, xt_d[sti, :, 16:32, :])
                for o in range(M):
                    ps = mm2.tile([128, 512], FP32, tag="mm")
                    for c in range(32):
                        xsrc = xa if c < 16 else xb
                        nc.tensor.matmul(ps[:, 0:256],
                                         wq_sb[:, c, ts(o, 128)],
                                         xsrc[:, c % 16, :],
                                         start=(c == 0), stop=(c == 31))
                    if o < 5:
                        nc.scalar.copy(qkvt[:, o, cslc], ps[:, 0:256])
                    else:
                        nc.scalar.copy(vstage[:, cslc], ps[:, 0:256])

        def emit_epilogue(t):
            qkvt = qkv_tiles.pop(t)
            vstage = vstages.pop(t)
            qkv_tiles2[t] = qkvt
            arouts[t] = None
            # v transpose: [d, kv] -> [kv, d]
            for bb in range(4):
                tps = scps.tile([128, 128], BF, tag="sc")
                nc.tensor.transpose(tps, vstage[:, ts(bb, 128)], ident)
                nc.vector.tensor_copy(v_sb[:, 4 * t + bb, :], tps)
            if stage < 2:
                return
            # sum of squares (weighted by 1/w^2) via ones-matmul
            sums = sumps.tile([128, 512], FP32)
            for sub in range(NSUB):
                cslc = slice(256 * sub, 256 * sub + 256)
                for cch in range(4):
                    sq = sqpool.tile([128, 256], BF, tag="sq")
                    nc.vector.tensor_mul(sq, qkvt[:, cch, cslc],
                                         qkvt[:, cch, cslc])
                    nc.tensor.matmul(sums[0:1, cslc], iw2[:, cch:cch + 1], sq,
                                     start=(cch == 0), stop=(cch == 3))
                sqk = sqpool.tile([128, 256], BF, tag="sq")
                nc.vector.tensor_mul(sqk, qkvt[:, 4, cslc], qkvt[:, 4, cslc])
                nc.tensor.matmul(sums[32:33, cslc], iw2[:, 4:5], sqk,
                                 start=True, stop=True)
            stg = stgpool.tile([33, 512], FP32, tag="stg")
            nc.vector.tensor_copy(stg[0:1, :], sums[0:1, :])
            nc.vector.tensor_copy(stg[32:33, :], sums[32:33, :])
            arin = dram.tile([2, 512], FP32, tag="arin")
            arout = dram.tile([2, 512], FP32, tag="arout")
            nc.gpsimd.dma_start(arin[0:1, :], stg[0:1, :])
            nc.gpsimd.dma_start(arin[1:2, :], stg[32:33, :])
            nc.gpsimd.collective_compute(
                "AllReduce", mybir.AluOpType.add,
                replica_groups=[list(range(NCORES))],
                ins=[arin.opt()], outs=[arout.opt()])
            arouts[t] = arout

        def emit_epilogue2(t):
            arout = arouts.pop(t)
            qkvt = qkv_tiles2.pop(t)
            if stage < 2:
                return
            rst = rcv.tile([1, 512], FP32, tag="rst")
            rsk = rcv.tile([1, 512], FP32, tag="rsk")
            nc.gpsimd.dma_start(rst, arout[0:1, :])
            nc.gpsimd.dma_start(rsk, arout[1:2, :])
            # rstd = 1/sqrt(mean + eps); mean divisor folded into scale
            nc.scalar.activation(rst, rst,
                                 mybir.ActivationFunctionType.Sqrt,
                                 bias=epsc[0:1, :], scale=1.0 / 4096.0)
            nc.scalar.activation(rsk, rsk,
                                 mybir.ActivationFunctionType.Sqrt,
                                 bias=epsc[0:1, :], scale=1.0 / 1024.0)
            nc.vector.reciprocal(rst, rst)
            nc.vector.reciprocal(rsk, rsk)
            # fold 1/sqrt(d) into the q-side factor
            nc.vector.tensor_scalar_mul(rst, rst, INV_SQRT_D)
            rqb = rqbp.tile([128, 512], FP32, tag="rqb")
            nc.gpsimd.partition_broadcast(rqb, rst)
            rkb = rqbp.tile([128, 512], FP32, tag="rkb")
            nc.gpsimd.partition_broadcast(rkb, rsk)
            if stage < 3:
                return
            # rope: out = x*cos + swap(x)*sn_signed (rotate-half sign folded
            # into the host sin table; swap(x) via PE permutation matmul).
            tc_sl = ts(t, 512)
            csx = cspool.tile([128, 512], BF, tag="cs")
            nc.sync.dma_start(csx, cs_d[:, tc_sl])
            snx = cspool.tile([128, 512], BF, tag="sn")
            nc.scalar.dma_start(snx, sn_d[:, tc_sl])
            for h in range(5):  # 4 q heads + k
                swps = mm2.tile([128, 512], FP32, tag="mm")
                nc.tensor.matmul(swps, pm, qkvt[:, h, :],
                                 start=True, stop=True)
                t1 = ropep.tile([128, 512], FP32, tag="t1")
                t2 = ropep.tile([128, 512], FP32, tag="t2")
                nc.vector.tensor_mul(t1, qkvt[:, h, :], csx)
                nc.vector.tensor_mul(t2, swps, snx)
                nc.vector.tensor_add(t1, t1, t2)
                if h < 4:
                    nc.vector.tensor_mul(qhat[:, h, tc_sl], t1, rqb)
                else:
                    nc.vector.tensor_mul(khat[:, tc_sl], t1, rkb)

        for t in range(NT + 2):
            if t < NT:
                emit_qkv(t)
            if 1 <= t <= NT:
                emit_epilogue(t - 1)
            if t >= 2:
                emit_epilogue2(t - 2)

        # ================= phase C: attention + proj =================
        # Per (g, h): all score matmuls first (exp latency hides behind the
        # later score blocks), then den/AV matmuls. Proj for t-pair p is
        # emitted after attention group 2p+3 so PE never waits on the
        # normalize chain.
        if stage < 4:
            return
        yhats = {}

        def emit_proj(p):
            yhat = yhats.pop(p)
            for o in range(32):
                ps = mm2.tile([128, 512], FP32, tag="mm")
                for cch in range(4):
                    nc.tensor.matmul(ps, wp_sb[:, cch, ts(o, 128)],
                                     yhat[:, cch, :],
                                     start=(cch == 0), stop=(cch == 3))
                ostg = ostgp.tile([128, 512], FP32, tag="ostg")
                if o % 2 == 0:
                    nc.scalar.copy(ostg, ps)
                else:
                    nc.vector.tensor_copy(ostg, ps)
                deng = nc.sync if o % 2 == 0 else nc.scalar
                deng.dma_start(yo_d[o, :, ts(p, 512)], ostg)

        for g in range(NG):
            if g % 2 == 0:
                yhat = yhatp.tile([128, HL, 512], BF, tag="yhat")
                yhats[g // 2] = yhat
            gc = slice(256 * (g % 2), 256 * (g % 2) + 256)
            jlo = max(0, 2 * g - 8)
            jhi = 2 * g + 1
            for h in range(HL):
                pts = {}
                for j in range(jlo, jhi + 1):
                    sp = scps.tile([128, 256], FP32, tag="sc")
                    nc.tensor.matmul(sp, khat[:, ts(j, 128)],
                                     qhat[:, h, ts(g, 256)],
                                     start=True, stop=True)
                    pt = ppool.tile([128, 256], BF, tag="p")
                    nc.scalar.activation(pt, sp,
                                         mybir.ActivationFunctionType.Exp,
                                         bias=zerob, scale=1.0)
                    if (j - 2 * g) in masks:
                        nc.vector.tensor_mul(pt, pt, masks[j - 2 * g])
                    pts[j] = pt
                yp = yps.tile([128, 256], FP32, tag="y")
                dp = dps.tile([1, 256], FP32, tag="d")
                for j in range(jlo, jhi + 1):
                    nc.tensor.matmul(dp, ones[:, 0:1], pts[j],
                                     start=(j == jlo), stop=(j == jhi))
                    nc.tensor.matmul(yp, v_sb[:, j, :], pts[j],
                                     start=(j == jlo), stop=(j == jhi))
                dsb = dsbp.tile([1, 256], FP32, tag="dsb")
                nc.vector.reciprocal(dsb, dp)
                drb = drbp.tile([128, 256], FP32, tag="drb")
                nc.gpsimd.partition_broadcast(drb, dsb)
                nc.vector.tensor_mul(yhat[:, h, gc], yp, drb)
            if stage >= 5 and g % 2 == 1 and g >= 3:
                emit_proj((g - 3) // 2)
        if stage >= 5:
            emit_proj(3)


_PROGRAM = None


def _get_program():
    global _PROGRAM
    if _PROGRAM is None:
        _PROGRAM = build_program()
    return _PROGRAM


def make_in_maps(x, cos, sin, W_qkv, norm_q_w, norm_k_w, W_proj):
    x2 = np.asarray(x, np.float32).reshape(T, C)
    xt = np.ascontiguousarray(
        x2.T.reshape(32, 128, 8, 256).transpose(2, 1, 0, 3)).astype(BF16)
    cs = np.ascontiguousarray(np.asarray(cos, np.float32).T).astype(BF16)
    sn_f = np.asarray(sin, np.float32).T.copy()  # [128, T]
    sn_f[0:64, :] *= -1.0  # rotate-half: lower half gets -x2*sin
    sn = np.ascontiguousarray(sn_f).astype(BF16)
    Wq = np.asarray(W_qkv, np.float32)
    Wp = np.asarray(W_proj, np.float32)
    nqw = np.asarray(norm_q_w, np.float32)
    nkw = np.asarray(norm_k_w, np.float32)
    pm = np.zeros((128, 128), np.float32)
    pm[np.arange(128), (np.arange(128) + 64) % 128] = 1.0
    pm = pm.astype(BF16)
    in_maps = []
    for i in range(NCORES):
        qs = slice(512 * i, 512 * i + 512)
        ks = slice(4096 + 128 * i, 4096 + 128 * i + 128)
        vs = slice(5120 + 128 * i, 5120 + 128 * i + 128)
        wq_rows = np.concatenate([
            Wq[qs] * nqw[qs][:, None],
            Wq[ks] * nkw[128 * i:128 * i + 128][:, None],
            Wq[vs],
        ], axis=0)  # [768, 4096]
        wq_t = np.ascontiguousarray(
            wq_rows.T.reshape(32, 128, 768).transpose(1, 0, 2)).astype(BF16)
        wp_t = np.ascontiguousarray(
            Wp[:, 512 * i:512 * i + 512].T.reshape(4, 128, C)
            .transpose(1, 0, 2)).astype(BF16)
        iw2 = np.ones((128, 8), np.float32)
        qw = nqw[qs].reshape(4, 128).T  # [p, chunk]
        kw = nkw[128 * i:128 * i + 128]
        with np.errstate(divide="ignore"):
            iw2[:, 0:4] = np.where(qw != 0.0, qw, 1.0) ** -2.0
            iw2[:, 4] = np.where(kw != 0.0, kw, 1.0) ** -2.0
        in_maps.append({
            "xt": xt, "cs": cs, "sn": sn,
            "wq": wq_t, "wp": wp_t, "iw2": iw2.astype(BF16), "pm": pm,
        })
    return in_maps


def combine_outputs(results):
    acc = np.zeros((C, T), np.float32)
    for r in results:
        acc += r["yo"].reshape(C, T)
    return np.ascontiguousarray(acc.T).reshape(1, T, C)


def kernel(x, cos, sin, W_qkv, norm_q_w, norm_k_w, W_proj):
    nc = _get_program()
    in_maps = make_in_maps(x, cos, sin, W_qkv, norm_q_w, norm_k_w, W_proj)
    res = bass_utils.run_bass_kernel_spmd(nc, in_maps,
                                          core_ids=list(range(NCORES)))
    return combine_outputs(res.results)



# revision 4
# speedup vs baseline: 1.2007x; 1.2007x over previous
"""Trainium2 Bass kernel for CausalSelfAttention (GQA + QK-RMSNorm + RoPE +
sliding-window causal attention + out-proj), tensor-parallel over 8 NeuronCores.

Sharding: core i owns q heads 4i..4i+3 and kv group i (split of the qkv output
dim and the proj input dim). The QK RMSNorm spans ALL heads, so per-core
partial sums of squares are combined with a tiny (2x512 f32 per t-tile)
AllReduce. The norm rstd is DEFERRED: rope runs unnormalized while the
collectives fly, and a cheap rescale pass applies rstd before attention, so
the PE never stalls on collective latency. A dummy collective at t=0 absorbs
the one-time mesh setup + core-skew wait. rstd = exp(-0.5*ln(mean+eps)) keeps
every activation in the single natural_log_exp table set (no table thrash
with the attention exps). Softmax denominators use the fast approx DVE
reciprocal. Proj partials are written bf16, partition-major, and summed on
the host.

Self-contained: hardcodes B=1, T=2048, C=4096, H=32, G=8, D=128, W=1024.
"""

import sys
import types
import numpy as np
import ml_dtypes

import concourse.bass as bass
import concourse.tile as tile
from concourse import bacc, mybir
from concourse import bass_utils
from concourse.bass import ts
from concourse.masks import make_identity

BF16 = ml_dtypes.bfloat16
FP32 = mybir.dt.float32
BF = mybir.dt.bfloat16

T = 2048          # tokens
C = 4096          # n_embd
D = 128           # head dim
HL = 4            # local q heads per core
M = 6             # local qkv out chunks of 128 (4 q + 1 k + 1 v)
NCORES = 8
EPS = 1e-5
NT = 4            # t-tiles of 512
NSUB = 2          # sub-tiles of 256 per t-tile
NG = 8            # q groups of 256
NKV = 16          # kv blocks of 128
LN_Q_BIAS = float(-3.5 * np.log(2.0))   # fold 1/sqrt(128) into exp bias


def _install_ntff_hook():
    """Re-register the axon NTFF profiling hook (the image lacks
    antenv.axon_hooks, so boot() degraded silently)."""
    if "antenv.axon_hooks" in sys.modules:
        return
    mod = types.ModuleType("antenv.axon_hooks")
    holder = [None]
    mod.set_axon_ntff_profile_hook = lambda h: holder.__setitem__(0, h)
    mod.get_axon_ntff_profile_hook = lambda: holder[0]
    sys.modules["antenv.axon_hooks"] = mod
    try:
        import antenv
        antenv.axon_hooks = mod
        from trn_agent_boot.trn_boot import _ntff_profile_via_ctypes
        mod.set_axon_ntff_profile_hook(
            _ntff_profile_via_ctypes("/opt/axon/libaxon_pjrt.so"))
    except Exception:
        pass


_install_ntff_hook()


def build_program():
    nc = bacc.Bacc("TRN2", target_bir_lowering=False, debug=False,
                   num_devices=NCORES)

    xt_d = nc.dram_tensor("xt", [8, 128, 32, 256], BF, kind="ExternalInput").ap()
    wq_d = nc.dram_tensor("wq", [128, 32, 768], BF, kind="ExternalInput").ap()
    wp_d = nc.dram_tensor("wp", [128, 4, C], BF, kind="ExternalInput").ap()
    cs_d = nc.dram_tensor("cs", [128, T], BF, kind="ExternalInput").ap()
    sn_d = nc.dram_tensor("sn", [128, T], BF, kind="ExternalInput").ap()
    iw2_d = nc.dram_tensor("iw2", [128, 8], BF, kind="ExternalInput").ap()
    pm_d = nc.dram_tensor("pm", [128, 128], BF, kind="ExternalInput").ap()
    yo_d = nc.dram_tensor("yo", [128, 32, T], BF, kind="ExternalOutput").ap()

    with tile.TileContext(nc) as tc:
        _emit(nc, tc, xt_d, wq_d, wp_d, cs_d, sn_d, iw2_d, pm_d, yo_d)
    nc.compile()
    return nc


def _emit(nc, tc, xt_d, wq_d, wp_d, cs_d, sn_d, iw2_d, pm_d, yo_d):
    import contextlib
    ctx = contextlib.ExitStack()
    with ctx:
        # ---------------- pools ----------------
        const = ctx.enter_context(tc.tile_pool(name="const", bufs=1))
        persist = ctx.enter_context(tc.tile_pool(name="persist", bufs=1))
        xpool = ctx.enter_context(tc.tile_pool(name="xpool", bufs=2))
        qkvpool = ctx.enter_context(tc.tile_pool(name="qkvpool", bufs=2))
        qkvcopy = ctx.enter_context(tc.tile_pool(name="qkvcopy", bufs=2))
        sqpool = ctx.enter_context(tc.tile_pool(name="sqpool", bufs=2))
        stgpool = ctx.enter_context(tc.tile_pool(name="stgpool", bufs=2))
        rcv = ctx.enter_context(tc.tile_pool(name="rcv", bufs=2))
        rqbp = ctx.enter_context(tc.tile_pool(name="rqbp", bufs=2))
        ropep = ctx.enter_context(tc.tile_pool(name="ropep", bufs=1))
        ppool = ctx.enter_context(tc.tile_pool(name="ppool", bufs=9))
        dsbp = ctx.enter_context(tc.tile_pool(name="dsbp", bufs=2))
        drbp = ctx.enter_context(tc.tile_pool(name="drbp", bufs=2))
        yhatp = ctx.enter_context(tc.tile_pool(name="yhatp", bufs=2))
        ostgp = ctx.enter_context(tc.tile_pool(name="ostgp", bufs=2))
        dram = ctx.enter_context(tc.tile_pool(name="dram", bufs=2, space="DRAM"))
        dram2 = ctx.enter_context(tc.tile_pool(name="dram2", bufs=1, space="DRAM"))

        # PSUM pools: 8 banks total
        mm2 = ctx.enter_context(tc.tile_pool(name="mm2", bufs=2, space="PSUM"))
        sumps = ctx.enter_context(tc.tile_pool(name="sumps", bufs=1, space="PSUM"))
        scps = ctx.enter_context(tc.tile_pool(name="scps", bufs=2, space="PSUM"))
        yps = ctx.enter_context(tc.tile_pool(name="yps", bufs=2, space="PSUM"))
        dps = ctx.enter_context(tc.tile_pool(name="dps", bufs=1, space="PSUM"))

        # ---------------- constants ----------------
        ident = const.tile([128, 128], BF)
        make_identity(nc, ident)
        ones = const.tile([128, 1], BF)
        nc.vector.memset(ones, 1.0)
        zerob = const.tile([128, 1], FP32)
        nc.vector.memset(zerob, 0.0)
        epsb = const.tile([1, 1], FP32)
        nc.vector.memset(epsb, EPS)
        cqb = const.tile([1, 1], FP32)
        nc.vector.memset(cqb, LN_Q_BIAS)
        zb1 = const.tile([1, 1], FP32)
        nc.vector.memset(zb1, 0.0)
        dummy = const.tile([1, 1], FP32)
        nc.vector.memset(dummy, 1.0)
        iw2 = const.tile([128, 8], BF)
        nc.sync.dma_start(iw2, iw2_d)
        pm = const.tile([128, 128], BF)
        nc.sync.dma_start(pm, pm_d)

        # Warm the activation table (ln+exp -> one combined set) at t=0.
        nc.scalar.activation(dummy, dummy,
                             mybir.ActivationFunctionType.Ln,
                             bias=epsb, scale=1.0)
        nc.scalar.activation(dummy, dummy,
                             mybir.ActivationFunctionType.Exp,
                             bias=zb1, scale=1.0)

        # Warm the collective path: a 1-element AllReduce absorbs the
        # one-time mesh setup + core launch skew while weights stream in.
        warm_in = dram2.tile([1, 1], FP32, tag="warm_in")
        warm_out = dram2.tile([1, 1], FP32, tag="warm_out")
        nc.gpsimd.dma_start(warm_in, zerob[0:1, 0:1])
        nc.gpsimd.collective_compute(
            "AllReduce", mybir.AluOpType.add,
            replica_groups=[list(range(NCORES))],
            ins=[warm_in.opt()], outs=[warm_out.opt()])

        # ---------------- persistent buffers ----------------
        wq_sb = persist.tile([128, 32, 768], BF)
        for h in range(4):
            nc.sync.dma_start(wq_sb[:, ts(h, 8), :], wq_d[:, ts(h, 8), :])
        wp_sb = persist.tile([128, 4, C], BF)
        for h in range(4):
            nc.scalar.dma_start(wp_sb[:, h, :], wp_d[:, h, :])
        cs_sb = persist.tile([128, T], BF)
        nc.sync.dma_start(cs_sb, cs_d)
        sn_sb = persist.tile([128, T], BF)
        nc.scalar.dma_start(sn_sb, sn_d)

        qhat = persist.tile([128, HL, T], BF)     # roped q (unnormed, then *=)
        khat = persist.tile([128, T], BF)         # roped k (unnormed, then *=)
        v_sb = persist.tile([128, NKV, 128], BF)  # v transposed [kv, d]

        # ========= phase A: qkv + epilogue (v-T, sumsq->AllReduce, rope) =====
        arouts = {}

        def emit_qkv(t):
            qkvt = qkvpool.tile([128, 5, 512], BF, tag="qkvt")
            vstage = qkvcopy.tile([128, 512], BF, tag="vstage")
            for sub in range(NSUB):
                cslc = slice(256 * sub, 256 * sub + 256)
                sti = 2 * t + sub
                xa = xpool.tile([128, 16, 256], BF, tag="xa")
                nc.sync.dma_start(xa, xt_d[sti, :, 0:16, :])
                xb = xpool.tile([128, 16, 256], BF, tag="xb")
                nc.scalar.dma_start(xb, xt_d[sti, :, 16:32, :])
                for o in range(M):
                    ps = mm2.tile([128, 512], FP32, tag="mm")
                    for c in range(32):
                        xsrc = xa if c < 16 else xb
                        nc.tensor.matmul(ps[:, 0:256],
                                         wq_sb[:, c, ts(o, 128)],
                                         xsrc[:, c % 16, :],
                                         start=(c == 0), stop=(c == 31))
                    if o < 5:
                        nc.scalar.copy(qkvt[:, o, cslc], ps[:, 0:256])
                    else:
                        nc.scalar.copy(vstage[:, cslc], ps[:, 0:256])
            return qkvt, vstage

        def emit_ep(t, qkvt, vstage):
            # v transpose: [d, kv] -> [kv, d]
            for bb in range(4):
                tps = scps.tile([128, 128], BF, tag="sc")
                nc.tensor.transpose(tps, vstage[:, ts(bb, 128)], ident)
                nc.vector.tensor_copy(v_sb[:, 4 * t + bb, :], tps)
            # sum of squares (weighted by 1/w^2) via ones-matmul
            sums = sumps.tile([128, 512], FP32)
            for sub in range(NSUB):
                cslc = slice(256 * sub, 256 * sub + 256)
                for cch in range(4):
                    sq = sqpool.tile([128, 256], BF, tag="sq")
                    nc.vector.tensor_mul(sq, qkvt[:, cch, cslc],
                                         qkvt[:, cch, cslc])
                    nc.tensor.matmul(sums[0:1, cslc], iw2[:, cch:cch + 1], sq,
                                     start=(cch == 0), stop=(cch == 3))
                sqk = sqpool.tile([128, 256], BF, tag="sq")
                nc.vector.tensor_mul(sqk, qkvt[:, 4, cslc], qkvt[:, 4, cslc])
                nc.tensor.matmul(sums[32:33, cslc], iw2[:, 4:5], sqk,
                                 start=True, stop=True)
            stg = stgpool.tile([33, 512], FP32, tag="stg")
            nc.vector.tensor_copy(stg[0:1, :], sums[0:1, :])
            nc.vector.tensor_copy(stg[32:33, :], sums[32:33, :])
            arin = dram.tile([2, 512], FP32, tag="arin")
            arout = dram.tile([2, 512], FP32, tag="arout")
            nc.gpsimd.dma_start(arin[0:1, :], stg[0:1, :])
            nc.gpsimd.dma_start(arin[1:2, :], stg[32:33, :])
            nc.gpsimd.collective_compute(
                "AllReduce", mybir.AluOpType.add,
                replica_groups=[list(range(NCORES))],
                ins=[arin.opt()], outs=[arout.opt()])
            arouts[t] = arout
            # rope (UNNORMED): out = x*cos + swap(x)*sn_signed; rstd applied
            # later in emit_rescale once the AllReduce lands.
            tc_sl = ts(t, 512)
            for h in range(5):  # 4 q heads + k
                swps = mm2.tile([128, 512], FP32, tag="mm")
                nc.tensor.matmul(swps, pm, qkvt[:, h, :],
                                 start=True, stop=True)
                t1 = ropep.tile([128, 512], FP32, tag="t1")
                t2 = ropep.tile([128, 512], FP32, tag="t2")
                nc.vector.tensor_mul(t1, qkvt[:, h, :], cs_sb[:, tc_sl])
                nc.vector.tensor_mul(t2, swps, sn_sb[:, tc_sl])
                if h < 4:
                    nc.vector.tensor_add(qhat[:, h, tc_sl], t1, t2)
                else:
                    nc.vector.tensor_add(khat[:, tc_sl], t1, t2)

        for t in range(NT):
            qkvt, vstage = emit_qkv(t)
            emit_ep(t, qkvt, vstage)

        # ========= phase B (interleaved into C): apply deferred rstd ========
        def emit_rescale(t):
            arout = arouts.pop(t)
            tc_sl = ts(t, 512)
            rsq = rcv.tile([1, 512], FP32, tag="rsq")
            rsk = rcv.tile([1, 512], FP32, tag="rsk")
            nc.gpsimd.dma_start(rsq, arout[0:1, :])
            nc.gpsimd.dma_start(rsk, arout[1:2, :])
            # rstd_q/sqrt(d) = exp(-0.5*ln(sum/4096 + eps) - 3.5*ln2)
            nc.scalar.activation(rsq, rsq,
                                 mybir.ActivationFunctionType.Ln,
                                 bias=epsb, scale=1.0 / 4096.0)
            nc.scalar.activation(rsq, rsq,
                                 mybir.ActivationFunctionType.Exp,
                                 bias=cqb, scale=-0.5)
            # rstd_k = exp(-0.5*ln(sum/1024 + eps))
            nc.scalar.activation(rsk, rsk,
                                 mybir.ActivationFunctionType.Ln,
                                 bias=epsb, scale=1.0 / 1024.0)
            nc.scalar.activation(rsk, rsk,
                                 mybir.ActivationFunctionType.Exp,
                                 bias=zb1, scale=-0.5)
            rqb = rqbp.tile([128, 512], FP32, tag="rqb")
            nc.gpsimd.partition_broadcast(rqb, rsq)
            rkb = rqbp.tile([128, 512], FP32, tag="rkb")
            nc.gpsimd.partition_broadcast(rkb, rsk)
            for h in range(4):
                nc.vector.tensor_mul(qhat[:, h, tc_sl], qhat[:, h, tc_sl], rqb)
            nc.vector.tensor_mul(khat[:, tc_sl], khat[:, tc_sl], rkb)

        # ================= phase C: attention + proj =================
        yhats = {}

        def emit_proj(p):
            yhat = yhats.pop(p)
            for oq in range(16):  # 2 o-chunks per DMA batch
                ostg = ostgp.tile([128, 2, 512], BF, tag="ostg")
                for oi in range(2):
                    o = 2 * oq + oi
                    ps = mm2.tile([128, 512], FP32, tag="mm")
                    for cch in range(4):
                        nc.tensor.matmul(ps, wp_sb[:, cch, ts(o, 128)],
                                         yhat[:, cch, :],
                                         start=(cch == 0), stop=(cch == 3))
                    if oi % 2 == 0:
                        nc.scalar.copy(ostg[:, oi, :], ps)
                    else:
                        nc.vector.tensor_copy(ostg[:, oi, :], ps)
                deng = nc.sync if oq % 2 == 0 else nc.scalar
                deng.dma_start(yo_d[:, ts(oq, 2), ts(p, 512)], ostg)

        for g in range(NG):
            if g % 2 == 0:
                emit_rescale(g // 2)
                yhat = yhatp.tile([128, HL, 512], BF, tag="yhat")
                yhats[g // 2] = yhat
            gc = slice(256 * (g % 2), 256 * (g % 2) + 256)
            jlo = max(0, 2 * g - 8)
            jhi = 2 * g + 1
            npair = (jhi - jlo + 1) // 2
            for h in range(HL):
                pts = {}
                for jp in range(npair):
                    j0 = jlo + 2 * jp
                    sp = scps.tile([128, 512], FP32, tag="sc")
                    for jj in range(2):
                        nc.tensor.matmul(sp[:, ts(jj, 256)],
                                         khat[:, ts(j0 + jj, 128)],
                                         qhat[:, h, ts(g, 256)],
                                         start=True, stop=True)
                    pt = ppool.tile([128, 512], BF, tag="p")
                    nc.scalar.activation(pt, sp,
                                         mybir.ActivationFunctionType.Exp,
                                         bias=zerob, scale=1.0)
                    for jj in range(2):
                        m = j0 + jj - 2 * g
                        if m in (0, 1):       # causal edge: keep d >= 0
                            nc.gpsimd.affine_select(
                                out=pt[:, ts(jj, 256)], in_=pt[:, ts(jj, 256)],
                                compare_op=mybir.AluOpType.is_ge,
                                fill=0.0, base=-128 * m, pattern=[[1, 256]],
                                channel_multiplier=-1)
                        elif m in (-7, -8):   # window edge: keep d <= 1023
                            nc.gpsimd.affine_select(
                                out=pt[:, ts(jj, 256)], in_=pt[:, ts(jj, 256)],
                                compare_op=mybir.AluOpType.is_ge,
                                fill=0.0, base=128 * m + 1023,
                                pattern=[[-1, 256]], channel_multiplier=1)
                        pts[j0 + jj] = pt[:, ts(jj, 256)]
                yp = yps.tile([128, 256], FP32, tag="y")
                dp = dps.tile([1, 256], FP32, tag="d")
                for j in range(jlo, jhi + 1):
                    nc.tensor.matmul(dp, ones[:, 0:1], pts[j],
                                     start=(j == jlo), stop=(j == jhi))
                    nc.tensor.matmul(yp, v_sb[:, j, :], pts[j],
                                     start=(j == jlo), stop=(j == jhi))
                dsb = dsbp.tile([1, 256], FP32, tag="dsb")
                nc.vector.reciprocal_approx_fast(out=dsb, in_=dp)
                drb = drbp.tile([128, 256], FP32, tag="drb")
                nc.gpsimd.partition_broadcast(drb, dsb)
                nc.vector.tensor_mul(yhat[:, h, gc], yp, drb)
            if g % 2 == 1 and g >= 3:
                emit_proj((g - 3) // 2)
        emit_proj(3)


_PROGRAM = None


def _get_program():
    global _PROGRAM
    if _PROGRAM is None:
        _PROGRAM = build_program()
    return _PROGRAM


def make_in_maps(x, cos, sin, W_qkv, norm_q_w, norm_k_w, W_proj):
    x2 = np.asarray(x, np.float32).reshape(T, C)
    xt = np.ascontiguousarray(
        x2.T.reshape(32, 128, 8, 256).transpose(2, 1, 0, 3)).astype(BF16)
    cs = np.ascontiguousarray(np.asarray(cos, np.float32).T).astype(BF16)
    sn_f = np.asarray(sin, np.float32).T.copy()  # [128, T]
    sn_f[0:64, :] *= -1.0  # rotate-half: lower half gets -x2*sin
    sn = np.ascontiguousarray(sn_f).astype(BF16)
    Wq = np.asarray(W_qkv, np.float32)
    Wp = np.asarray(W_proj, np.float32)
    nqw = np.asarray(norm_q_w, np.float32)
    nkw = np.asarray(norm_k_w, np.float32)
    pm = np.zeros((128, 128), np.float32)
    pm[np.arange(128), (np.arange(128) + 64) % 128] = 1.0
    pm = pm.astype(BF16)
    in_maps = []
    for i in range(NCORES):
        qs = slice(512 * i, 512 * i + 512)
        ks = slice(4096 + 128 * i, 4096 + 128 * i + 128)
        vs = slice(5120 + 128 * i, 5120 + 128 * i + 128)
        wq_rows = np.concatenate([
            Wq[qs] * nqw[qs][:, None],
            Wq[ks] * nkw[128 * i:128 * i + 128][:, None],
            Wq[vs],
        ], axis=0)  # [768, 4096]
        wq_t = np.ascontiguousarray(
            wq_rows.T.reshape(32, 128, 768).transpose(1, 0, 2)).astype(BF16)
        wp_t = np.ascontiguousarray(
            Wp[:, 512 * i:512 * i + 512].T.reshape(4, 128, C)
            .transpose(1, 0, 2)).astype(BF16)
        iw2 = np.ones((128, 8), np.float32)
        qw = nqw[qs].reshape(4, 128).T  # [p, chunk]
        kw = nkw[128 * i:128 * i + 128]
        with np.errstate(divide="ignore"):
            iw2[:, 0:4] = np.where(qw != 0.0, qw, 1.0) ** -2.0
            iw2[:, 4] = np.where(kw != 0.0, kw, 1.0) ** -2.0
        in_maps.append({
            "xt": xt, "cs": cs, "sn": sn,
            "wq": wq_t, "wp": wp_t, "iw2": iw2.astype(BF16), "pm": pm,
        })
    return in_maps


def combine_outputs(results):
    acc = np.zeros((128, 32, T), np.float32)
    for r in results:
        acc += np.asarray(r["yo"], dtype=np.float32)
    # yo[p, o, t] -> out[t, o*128 + p]
    return np.ascontiguousarray(
        acc.transpose(1, 0, 2).reshape(C, T).T).reshape(1, T, C)


def kernel(x, cos, sin, W_qkv, norm_q_w, norm_k_w, W_proj):
    nc = _get_program()
    in_maps = make_in_maps(x, cos, sin, W_qkv, norm_q_w, norm_k_w, W_proj)
    res = bass_utils.run_bass_kernel_spmd(nc, in_maps,
                                          core_ids=list(range(NCORES)))
    return combine_outputs(res.results)
